# revision 38
# baseline (speedup 1.0000x reference)
"""GPTQ 4-bit quantized linear (nn_Ex4bitLinear) for 8 Trainium2 NeuronCores.

Computes out = x @ dequant(qweight, qzeros, scales) + bias where
  x:       [8192, 4096] fp32
  qweight: [512, 4096] int32 (8 x 4-bit along K per word)
  qzeros:  [32, 512] int32 (8 x 4-bit along N per word)
  scales:  [32, 4096] fp32, groupsize 128 (standard g_idx = k // 128)
  bias:    [4096] fp32

Sharding: 2-way on tokens x 4-way on out-features (core = ti*4 + ni).
Per core: T_s=4096 tokens, K=4096, N_s=1024. Single-pass bf16 matmuls
(fp8 DoubleRow was measured 2x/instruction but needs 3 logical passes
for the 2e-2 gate -> 1.5x slower than bf16's one pass).

Structure:
- Zero-point/bias folding: W = s*q - zs with zs[k,n] = (z+1)*s constant
  per 128-k quant group, so x @ zs = XS @ ZS is RANK-32: XS[t,g] =
  group-sums of x (computed host-side, padded with a ones column for
  bias). One extra 128-contraction bf16 matmul per PSUM group replaces
  the per-chunk dequant subtract, the zero/bias broadcast DMAs, and the
  output bias add; the PSUM result IS the final output and is DMA'd to
  DRAM directly.
- k-permutation: virtual chunk cv = 8*pt + j holds real k = 1024*pt +
  8*r + j at partition r, so the device 4-bit unpack is ONE fused
  shift+mask tensor_scalar per nibble over a whole packed qweight tile,
  and quant groups stay partition-aligned (group = 8*pt + r//16) for
  one broadcast scale-DMA per packed tile. Dequant is then 2 DVE ops
  per chunk (unpack, scale-mult -> bf16 W), production rate ~2.1us per
  chunk, matching PE consumption at 4 in-flight token tiles.
- x is pre-cast to bf16 and pre-tiled host-side into the k-permuted
  SBUF layout: device x-path is one contiguous DMA per token tile.
- Nothing runs on Pool/GpSimd (software Q7 engine, 0.42 efficiency) and
  all DMAs use hardware DGE queues (sync: x, scalar: rest).
"""

import numpy as np
from contextlib import ExitStack

import ml_dtypes
import concourse.bass as bass
import concourse.mybir as mybir
import concourse.tile as tile
from concourse import bacc
from concourse.bass_utils import run_bass_kernel_spmd

P = 128
GROUPSIZE = 128

TOKENS_F, K_F, N_F = 8192, 4096, 4096
TSHARD, NSHARD = 2, 4
N_CORES = TSHARD * NSHARD
XTP_BUFS = 8
XSP_BUFS = 4
OP_BUFS = 4
PSUM_BUFS = 8          # [128, 512] fp32 tiles, 1 bank each (ISA limit)
BF16 = ml_dtypes.bfloat16


def build_kernel(T_s=TOKENS_F // TSHARD, K=K_F, N_s=N_F // NSHARD,
                 no_dequant=False, no_xpath=False, no_matmul=False, reps=1):
    """Per-core Bass program. reps>1 wraps the body in a For_i loop for
    R-slope timing."""
    assert T_s % P == 0 and K % 1024 == 0 and N_s % 512 == 0
    C = K // P                  # virtual 128-deep contraction chunks
    PT = K // 1024              # packed qweight row-tiles (8 chunks each)
    TT = T_s // P

    nc = bacc.Bacc("TRN2", target_bir_lowering=False, debug=False)
    xb_d = nc.dram_tensor("xb", [T_s, K], mybir.dt.bfloat16,
                          kind="ExternalInput")
    xse_d = nc.dram_tensor("xse", [T_s, P], mybir.dt.bfloat16,
                           kind="ExternalInput")   # [tt, g, t] tiles
    qw_d = nc.dram_tensor("qw", [K // 8, N_s], mybir.dt.int32,
                          kind="ExternalInput")
    sc_d = nc.dram_tensor("sc", [C, N_s], mybir.dt.float32,
                          kind="ExternalInput")    # scales
    rhse_d = nc.dram_tensor("rhse", [P, N_s], mybir.dt.bfloat16,
                            kind="ExternalInput")  # [-zs rows; bias; 0...]
    out_d = nc.dram_tensor("out", [T_s, N_s], mybir.dt.float32,
                           kind="ExternalOutput")

    with tile.TileContext(nc) as tc, ExitStack() as ctx:
        const = ctx.enter_context(tc.tile_pool(name="const", bufs=1))
        wp = ctx.enter_context(tc.tile_pool(name="wp", bufs=C))
        dq = ctx.enter_context(tc.tile_pool(name="dq", bufs=2))
        xtp = ctx.enter_context(tc.tile_pool(name="xtp", bufs=XTP_BUFS))
        xsp = ctx.enter_context(tc.tile_pool(name="xsp", bufs=XSP_BUFS))
        op = ctx.enter_context(tc.tile_pool(name="op", bufs=OP_BUFS))
        psum = ctx.enter_context(tc.tile_pool(name="psum", bufs=PSUM_BUFS,
                                              space="PSUM"))

        def body():
            # ---- dequantize W' = q*s into bf16 tiles, virtual-chunk order --
            # Emission order = engine-queue order: scale/qweight DMAs go
            # first on the scalar queue so chunk 0 exists ASAP; rhse (not
            # needed until the first tile closes) loads after them.
            w_tiles = [None] * C
            s_ts = []
            if no_dequant:
                for cv in range(C):
                    w = wp.tile([P, N_s], mybir.dt.bfloat16, tag="w")
                    nc.vector.memset(w[:], 0.25)
                    w_tiles[cv] = w
            else:
                for pt in range(PT):
                    ap = [[N_s, 8], [0, 16], [1, N_s]]
                    s_t = const.tile([P, N_s], mybir.dt.float32,
                                     name=f"s{pt}")
                    nc.scalar.dma_start(s_t[:],
                                        bass.AP(sc_d, pt * 8 * N_s, ap))
                    s_ts.append(s_t)
                    qb = dq.tile([P, N_s], mybir.dt.int32, tag="qb")
                    nc.scalar.dma_start(qb[:], qw_d[pt * P:(pt + 1) * P, :])
                    for j in range(8):
                        cv = 8 * pt + j
                        qj = dq.tile([P, N_s], mybir.dt.int32, tag="qj")
                        nc.vector.tensor_scalar(
                            qj[:], qb[:], 4 * j, 0xF,
                            mybir.AluOpType.logical_shift_right,
                            mybir.AluOpType.bitwise_and,
                        )
                        w = wp.tile([P, N_s], mybir.dt.bfloat16, tag="w")
                        nc.vector.tensor_tensor(
                            w[:], qj[:], s_ts[pt][:], mybir.AluOpType.mult)
                        w_tiles[cv] = w

            rhse = const.tile([P, N_s], mybir.dt.bfloat16)
            nc.scalar.dma_start(rhse[:], rhse_d[:, :])

            # ---- matmuls ----
            # The PE queue is in-order: during dequant, emit the first
            # PIPE tiles chunk-major so PE consumption (PIPE*NB*250ns per
            # chunk) matches dequant production (~2.1us per chunk) with no
            # head-of-line stall; remaining tiles go tile-major at full
            # speed with all W resident.
            NB = N_s // 512
            PIPE = min(PSUM_BUFS // NB, TT)

            tiles = {}

            def open_tile(t):
                xt = xtp.tile([P, C, P], mybir.dt.bfloat16, tag="xt",
                              name=f"xt{t}")
                xs = xsp.tile([P, P], mybir.dt.bfloat16, tag="xs",
                              name=f"xs{t}")
                if no_xpath:
                    nc.vector.memset(xt[:], 0.25)
                    nc.vector.memset(xs[:], 0.25)
                else:
                    # quarter-tile x loads: chunk 0's matmul can start after
                    # 256KB instead of the full 1MB (subtile deps)
                    CQ = C // 4
                    for q in range(4):
                        nc.sync.dma_start(
                            xt[:, q * CQ:(q + 1) * CQ, :],
                            xb_d[t * P:(t + 1) * P,
                                 q * CQ * P:(q + 1) * CQ * P])
                    nc.scalar.dma_start(xs[:], xse_d[t * P:(t + 1) * P, :])
                psums = [psum.tile([P, 512], mybir.dt.float32, tag="ps",
                                   name=f"ps{nb}") for nb in range(NB)]
                tiles[t] = (xt, xs, psums)
                if no_matmul:
                    for ps in psums:
                        nc.vector.memset(ps[:], 0.0)

            def chunk_mms(t, cv):
                xt, _, psums = tiles[t]
                lhsT = xt[:, cv, :]
                for nb in range(NB):
                    nc.tensor.matmul(
                        psums[nb][:], lhsT,
                        w_tiles[cv][:, nb * 512:(nb + 1) * 512],
                        start=(cv == 0), stop=False,
                    )

            def close_tile(t):
                _, xs, psums = tiles.pop(t)
                for nb in range(NB):
                    if not no_matmul:
                        # rank-128 zero-point+bias term closes the group;
                        # its xs load has a whole tile of slack.
                        nc.tensor.matmul(
                            psums[nb][:], xs[:],
                            rhse[:, nb * 512:(nb + 1) * 512],
                            start=False, stop=True,
                        )
                    o = op.tile([P, 512], mybir.dt.float32, tag="o")
                    nc.vector.tensor_copy(o[:], psums[nb][:])
                    nc.scalar.dma_start(
                        out_d[t * P:(t + 1) * P, nb * 512:(nb + 1) * 512],
                        o[:])

            for t in range(PIPE):
                open_tile(t)
            if not no_matmul:
                for cv in range(C):              # chunk-major warm-up wave
                    for t in range(PIPE):
                        chunk_mms(t, cv)
            for t in range(PIPE):
                close_tile(t)
            for t in range(PIPE, TT):            # steady state, tile-major
                open_tile(t)
                if not no_matmul:
                    for cv in range(C):
                        chunk_mms(t, cv)
                close_tile(t)

        if reps == 1:
            body()
        else:
            with tc.For_i(0, reps, 1):
                body()

    nc.compile()
    return nc


_cache = {}


def _get_kernel(T_s, K, N_s):
    key = (T_s, K, N_s)
    if key not in _cache:
        _cache[key] = build_kernel(T_s, K, N_s)
    return _cache[key]


def _tile_x(xb, TT, PT):
    """[T_s, K] bf16 -> blob[tt, r, pt, j, t] matching the device layout:
    virtual chunk cv=8*pt+j, partition r holds real k = 1024*pt + 8*r + j."""
    T_s, K = xb.shape
    v = xb.reshape(TT, P, PT, P, 8)            # [tt, t, pt, r, j]
    return v.transpose(0, 3, 2, 4, 1)          # [tt, r, pt, j, t]


def make_in_maps(x, qweight, qzeros, scales, bias):
    """Split full inputs into per-core dicts (2 token x 4 feature shards).

    Host prep: x cast to bf16 and pre-tiled into the k-permuted SBUF
    layout; group-sums XS (+ones column) pre-transposed per token tile;
    qzeros unpacked and folded with bias into the rank-33 RHS matrix."""
    t_sz = x.shape[0] // TSHARD
    n_sz = qweight.shape[1] // NSHARD
    K = x.shape[1]
    G = K // GROUPSIZE
    TT, PT = t_sz // P, K // 1024

    x = x.astype(np.float32)
    xb = x.astype(BF16)
    # group sums + ones column, padded to 128, pre-transposed per tile
    xs = x.reshape(x.shape[0], G, GROUPSIZE).sum(axis=2)      # [T, G]
    xse = np.zeros((x.shape[0], P), dtype=np.float32)
    xse[:, :G] = xs
    xse[:, G] = 1.0
    xse = xse.astype(BF16)

    shifts = (np.arange(8, dtype=np.int32) * 4)
    z = ((qzeros[:, :, None] >> shifts[None, None, :]) & 0xF).reshape(
        qzeros.shape[0], -1)
    zs = ((z + 1).astype(np.float32) * scales).astype(np.float32)  # [G, N]
    rhse_full = np.zeros((P, qweight.shape[1]), dtype=np.float32)
    rhse_full[:G] = -zs
    rhse_full[G] = bias
    rhse_full = rhse_full.astype(BF16)

    xblobs, xseblobs = [], []
    for ti in range(TSHARD):
        blob = _tile_x(xb[ti * t_sz:(ti + 1) * t_sz], TT, PT)
        xblobs.append(np.ascontiguousarray(blob.reshape(t_sz, K)))
        # [tt, t, g] -> [tt, g, t] so partitions are g
        xv = xse[ti * t_sz:(ti + 1) * t_sz].reshape(TT, P, P)
        xseblobs.append(np.ascontiguousarray(
            xv.transpose(0, 2, 1).reshape(t_sz, P)))

    in_maps = []
    for core in range(N_CORES):
        ti, ni = divmod(core, NSHARD)
        in_maps.append({
            "xb": xblobs[ti],
            "xse": xseblobs[ti],
            "qw": np.ascontiguousarray(qweight[:, ni * n_sz:(ni + 1) * n_sz]),
            "sc": np.ascontiguousarray(scales[:, ni * n_sz:(ni + 1) * n_sz]),
            "rhse": np.ascontiguousarray(rhse_full[:, ni * n_sz:(ni + 1) * n_sz]),
        })
    return in_maps


def assemble(results, tokens, n):
    t_sz = tokens // TSHARD
    n_sz = n // NSHARD
    out = np.empty((tokens, n), dtype=np.float32)
    for core in range(N_CORES):
        ti, ni = divmod(core, NSHARD)
        out[ti * t_sz:(ti + 1) * t_sz, ni * n_sz:(ni + 1) * n_sz] = \
            results[core]["out"]
    return out


def kernel(x, qweight, qzeros, scales, g_idx, bias, _trace=False):
    x = np.asarray(x, dtype=np.float32)
    qweight = np.asarray(qweight, dtype=np.int32)
    qzeros = np.asarray(qzeros, dtype=np.int32)
    scales = np.asarray(scales, dtype=np.float32)
    bias = np.asarray(bias, dtype=np.float32)

    nc = _get_kernel(x.shape[0] // TSHARD, x.shape[1],
                     qweight.shape[1] // NSHARD)
    in_maps = make_in_maps(x, qweight, qzeros, scales, bias)
    res = run_bass_kernel_spmd(
        nc, in_maps, core_ids=list(range(N_CORES)), trace=_trace,
    )
    out = assemble(res.results, x.shape[0], qweight.shape[1])
    if _trace:
        kernel.last_result = res
    return out


# revision 39
# speedup vs baseline: 1.0357x; 1.0357x over previous
"""GPTQ 4-bit quantized linear (nn_Ex4bitLinear) for 8 Trainium2 NeuronCores.

Computes out = x @ dequant(qweight, qzeros, scales) + bias where
  x:       [8192, 4096] fp32
  qweight: [512, 4096] int32 (8 x 4-bit along K per word)
  qzeros:  [32, 512] int32 (8 x 4-bit along N per word)
  scales:  [32, 4096] fp32, groupsize 128 (standard g_idx = k // 128)
  bias:    [4096] fp32

Sharding: 2-way on tokens x 4-way on out-features (core = ti*4 + ni).
Per core: T_s=4096 tokens, K=4096, N_s=1024. Single-pass bf16 matmuls
(fp8 DoubleRow was measured 2x/instruction but needs 3 logical passes
for the 2e-2 gate -> 1.5x slower than bf16's one pass).

Structure:
- Zero-point/bias folding: W = s*q - zs with zs[k,n] = (z+1)*s constant
  per 128-k quant group, so x @ zs = XS @ ZS is RANK-32: XS[t,g] =
  group-sums of x (computed host-side, padded with a ones column for
  bias). One extra 128-contraction bf16 matmul per PSUM group replaces
  the per-chunk dequant subtract, the zero/bias broadcast DMAs, and the
  output bias add; the PSUM result IS the final output and is DMA'd to
  DRAM directly.
- k-permutation: virtual chunk cv = 8*pt + j holds real k = 1024*pt +
  8*r + j at partition r, so the device 4-bit unpack is ONE fused
  shift+mask tensor_scalar per nibble over a whole packed qweight tile,
  and quant groups stay partition-aligned (group = 8*pt + r//16) for
  one broadcast scale-DMA per packed tile. Dequant is then 2 DVE ops
  per chunk (unpack, scale-mult -> bf16 W), production rate ~2.1us per
  chunk, matching PE consumption at 4 in-flight token tiles.
- x is pre-cast to bf16 and pre-tiled host-side into the k-permuted
  SBUF layout: device x-path is four contiguous quarter-tile DMAs per
  token tile, so chunk-0 matmuls start after 256KB (subtile deps).
- Nothing runs on Pool/GpSimd (software Q7 engine, 0.42 efficiency) and
  all DMAs use hardware DGE queues (sync: x, scalar: rest).

NeuronCore-v3 ISA note: double_row (fp8-only) is the sole matmul
performance mode; bf16 single-pass at ~250ns per 512-row instruction is
the PE roofline for this problem at the 2e-2 accuracy gate.
"""

import numpy as np
from contextlib import ExitStack

import ml_dtypes
import concourse.bass as bass
import concourse.mybir as mybir
import concourse.tile as tile
from concourse import bacc
from concourse.bass_utils import run_bass_kernel_spmd

P = 128
GROUPSIZE = 128

TOKENS_F, K_F, N_F = 8192, 4096, 4096
TSHARD, NSHARD = 2, 4
N_CORES = TSHARD * NSHARD
XTP_BUFS = 8
XSP_BUFS = 4
OP_BUFS = 4
PSUM_BUFS = 8          # [128, 512] fp32 tiles, 1 bank each (ISA limit)
BF16 = ml_dtypes.bfloat16


def build_kernel(T_s=TOKENS_F // TSHARD, K=K_F, N_s=N_F // NSHARD,
                 no_dequant=False, no_xpath=False, no_matmul=False, reps=1):
    """Per-core Bass program. reps>1 wraps the body in a For_i loop for
    R-slope timing."""
    assert T_s % P == 0 and K % 1024 == 0 and N_s % 512 == 0
    C = K // P                  # virtual 128-deep contraction chunks
    PT = K // 1024              # packed qweight row-tiles (8 chunks each)
    TT = T_s // P

    nc = bacc.Bacc("TRN2", target_bir_lowering=False, debug=False)
    xb_d = nc.dram_tensor("xb", [T_s, K], mybir.dt.bfloat16,
                          kind="ExternalInput")
    xse_d = nc.dram_tensor("xse", [T_s, P], mybir.dt.bfloat16,
                           kind="ExternalInput")   # [tt, g, t] tiles
    qw_d = nc.dram_tensor("qw", [K // 8, N_s], mybir.dt.int32,
                          kind="ExternalInput")
    sc_d = nc.dram_tensor("sc", [C, N_s], mybir.dt.float32,
                          kind="ExternalInput")    # scales
    rhse_d = nc.dram_tensor("rhse", [P, N_s], mybir.dt.bfloat16,
                            kind="ExternalInput")  # [-zs rows; bias; 0...]
    out_d = nc.dram_tensor("out", [T_s, N_s], mybir.dt.float32,
                           kind="ExternalOutput")

    with tile.TileContext(nc) as tc, ExitStack() as ctx:
        const = ctx.enter_context(tc.tile_pool(name="const", bufs=1))
        wp = ctx.enter_context(tc.tile_pool(name="wp", bufs=C))
        dq = ctx.enter_context(tc.tile_pool(name="dq", bufs=2))
        xtp = ctx.enter_context(tc.tile_pool(name="xtp", bufs=XTP_BUFS))
        xsp = ctx.enter_context(tc.tile_pool(name="xsp", bufs=XSP_BUFS))
        op = ctx.enter_context(tc.tile_pool(name="op", bufs=OP_BUFS))
        psum = ctx.enter_context(tc.tile_pool(name="psum", bufs=PSUM_BUFS,
                                              space="PSUM"))

        def body():
            # ---- dequantize W' = q*s into bf16 tiles, virtual-chunk order --
            # Emission order = engine-queue order: scale/qweight DMAs go
            # first on the scalar queue so chunk 0 exists ASAP; rhse (not
            # needed until the first tile closes) loads after them.
            w_tiles = [None] * C
            s_ts = []
            if no_dequant:
                for cv in range(C):
                    w = wp.tile([P, N_s], mybir.dt.bfloat16, tag="w")
                    nc.vector.memset(w[:], 0.25)
                    w_tiles[cv] = w
            else:
                for pt in range(PT):
                    ap = [[N_s, 8], [0, 16], [1, N_s]]
                    s_t = const.tile([P, N_s], mybir.dt.float32,
                                     name=f"s{pt}")
                    nc.scalar.dma_start(s_t[:],
                                        bass.AP(sc_d, pt * 8 * N_s, ap))
                    s_ts.append(s_t)
                    qb = dq.tile([P, N_s], mybir.dt.int32, tag="qb")
                    nc.scalar.dma_start(qb[:], qw_d[pt * P:(pt + 1) * P, :])
                    for j in range(8):
                        cv = 8 * pt + j
                        qj = dq.tile([P, N_s], mybir.dt.int32, tag="qj")
                        nc.vector.tensor_scalar(
                            qj[:], qb[:], 4 * j, 0xF,
                            mybir.AluOpType.logical_shift_right,
                            mybir.AluOpType.bitwise_and,
                        )
                        w = wp.tile([P, N_s], mybir.dt.bfloat16, tag="w")
                        nc.vector.tensor_tensor(
                            w[:], qj[:], s_ts[pt][:], mybir.AluOpType.mult)
                        w_tiles[cv] = w

            rhse = const.tile([P, N_s], mybir.dt.bfloat16)
            nc.scalar.dma_start(rhse[:], rhse_d[:, :])

            # ---- matmuls ----
            # The PE queue is in-order: during dequant, emit the first
            # PIPE tiles chunk-major so PE consumption (PIPE*NB*250ns per
            # chunk) matches dequant production (~2.1us per chunk) with no
            # head-of-line stall; remaining tiles go tile-major at full
            # speed with all W resident.
            NB = N_s // 512
            PIPE = min(PSUM_BUFS // NB, TT)

            tiles = {}

            def open_tile(t):
                xt = xtp.tile([P, C, P], mybir.dt.bfloat16, tag="xt",
                              name=f"xt{t}")
                xs = xsp.tile([P, P], mybir.dt.bfloat16, tag="xs",
                              name=f"xs{t}")
                if no_xpath:
                    nc.vector.memset(xt[:], 0.25)
                    nc.vector.memset(xs[:], 0.25)
                else:
                    # quarter-tile x loads: chunk 0's matmul can start after
                    # 256KB instead of the full 1MB (subtile deps)
                    CQ = C // 4
                    for q in range(4):
                        nc.sync.dma_start(
                            xt[:, q * CQ:(q + 1) * CQ, :],
                            xb_d[t * P:(t + 1) * P,
                                 q * CQ * P:(q + 1) * CQ * P])
                    nc.scalar.dma_start(xs[:], xse_d[t * P:(t + 1) * P, :])
                psums = [psum.tile([P, 512], mybir.dt.float32, tag="ps",
                                   name=f"ps{nb}") for nb in range(NB)]
                tiles[t] = (xt, xs, psums)
                if no_matmul:
                    for ps in psums:
                        nc.vector.memset(ps[:], 0.0)

            def chunk_mms(t, cv):
                xt, _, psums = tiles[t]
                lhsT = xt[:, cv, :]
                for nb in range(NB):
                    nc.tensor.matmul(
                        psums[nb][:], lhsT,
                        w_tiles[cv][:, nb * 512:(nb + 1) * 512],
                        start=(cv == 0), stop=False,
                    )

            def close_tile(t):
                _, xs, psums = tiles.pop(t)
                for nb in range(NB):
                    if not no_matmul:
                        # rank-128 zero-point+bias term closes the group;
                        # its xs load has a whole tile of slack.
                        nc.tensor.matmul(
                            psums[nb][:], xs[:],
                            rhse[:, nb * 512:(nb + 1) * 512],
                            start=False, stop=True,
                        )
                    o = op.tile([P, 512], mybir.dt.float32, tag="o")
                    nc.vector.tensor_copy(o[:], psums[nb][:])
                    nc.scalar.dma_start(
                        out_d[t * P:(t + 1) * P, nb * 512:(nb + 1) * 512],
                        o[:])

            for t in range(PIPE):
                open_tile(t)
            if not no_matmul:
                for cv in range(C):              # chunk-major warm-up wave
                    for t in range(PIPE):
                        chunk_mms(t, cv)
            for t in range(PIPE):
                close_tile(t)
            for t in range(PIPE, TT):            # steady state, tile-major
                open_tile(t)
                if not no_matmul:
                    for cv in range(C):
                        chunk_mms(t, cv)
                close_tile(t)

        if reps == 1:
            body()
        else:
            with tc.For_i(0, reps, 1):
                body()

    nc.compile()
    return nc


_cache = {}


def _get_kernel(T_s, K, N_s):
    key = (T_s, K, N_s)
    if key not in _cache:
        _cache[key] = build_kernel(T_s, K, N_s)
    return _cache[key]


def _tile_x(xb, TT, PT):
    """[T_s, K] bf16 -> blob[tt, r, pt, j, t] matching the device layout:
    virtual chunk cv=8*pt+j, partition r holds real k = 1024*pt + 8*r + j."""
    T_s, K = xb.shape
    v = xb.reshape(TT, P, PT, P, 8)            # [tt, t, pt, r, j]
    return v.transpose(0, 3, 2, 4, 1)          # [tt, r, pt, j, t]


def make_in_maps(x, qweight, qzeros, scales, bias):
    """Split full inputs into per-core dicts (2 token x 4 feature shards).

    Host prep: x cast to bf16 and pre-tiled into the k-permuted SBUF
    layout; group-sums XS (+ones column) pre-transposed per token tile;
    qzeros unpacked and folded with bias into the rank-33 RHS matrix."""
    t_sz = x.shape[0] // TSHARD
    n_sz = qweight.shape[1] // NSHARD
    K = x.shape[1]
    G = K // GROUPSIZE
    TT, PT = t_sz // P, K // 1024

    x = x.astype(np.float32)
    xb = x.astype(BF16)
    # group sums + ones column, padded to 128, pre-transposed per tile
    xs = x.reshape(x.shape[0], G, GROUPSIZE).sum(axis=2)      # [T, G]
    xse = np.zeros((x.shape[0], P), dtype=np.float32)
    xse[:, :G] = xs
    xse[:, G] = 1.0
    xse = xse.astype(BF16)

    shifts = (np.arange(8, dtype=np.int32) * 4)
    z = ((qzeros[:, :, None] >> shifts[None, None, :]) & 0xF).reshape(
        qzeros.shape[0], -1)
    zs = ((z + 1).astype(np.float32) * scales).astype(np.float32)  # [G, N]
    rhse_full = np.zeros((P, qweight.shape[1]), dtype=np.float32)
    rhse_full[:G] = -zs
    rhse_full[G] = bias
    rhse_full = rhse_full.astype(BF16)

    xblobs, xseblobs = [], []
    for ti in range(TSHARD):
        blob = _tile_x(xb[ti * t_sz:(ti + 1) * t_sz], TT, PT)
        xblobs.append(np.ascontiguousarray(blob.reshape(t_sz, K)))
        # [tt, t, g] -> [tt, g, t] so partitions are g
        xv = xse[ti * t_sz:(ti + 1) * t_sz].reshape(TT, P, P)
        xseblobs.append(np.ascontiguousarray(
            xv.transpose(0, 2, 1).reshape(t_sz, P)))

    in_maps = []
    for core in range(N_CORES):
        ti, ni = divmod(core, NSHARD)
        in_maps.append({
            "xb": xblobs[ti],
            "xse": xseblobs[ti],
            "qw": np.ascontiguousarray(qweight[:, ni * n_sz:(ni + 1) * n_sz]),
            "sc": np.ascontiguousarray(scales[:, ni * n_sz:(ni + 1) * n_sz]),
            "rhse": np.ascontiguousarray(rhse_full[:, ni * n_sz:(ni + 1) * n_sz]),
        })
    return in_maps


def assemble(results, tokens, n):
    t_sz = tokens // TSHARD
    n_sz = n // NSHARD
    out = np.empty((tokens, n), dtype=np.float32)
    for core in range(N_CORES):
        ti, ni = divmod(core, NSHARD)
        out[ti * t_sz:(ti + 1) * t_sz, ni * n_sz:(ni + 1) * n_sz] = \
            results[core]["out"]
    return out


def kernel(x, qweight, qzeros, scales, g_idx, bias, _trace=False):
    x = np.asarray(x, dtype=np.float32)
    qweight = np.asarray(qweight, dtype=np.int32)
    qzeros = np.asarray(qzeros, dtype=np.int32)
    scales = np.asarray(scales, dtype=np.float32)
    bias = np.asarray(bias, dtype=np.float32)

    nc = _get_kernel(x.shape[0] // TSHARD, x.shape[1],
                     qweight.shape[1] // NSHARD)
    in_maps = make_in_maps(x, qweight, qzeros, scales, bias)
    res = run_bass_kernel_spmd(
        nc, in_maps, core_ids=list(range(N_CORES)), trace=_trace,
    )
    out = assemble(res.results, x.shape[0], qweight.shape[1])
    if _trace:
        kernel.last_result = res
    return out
